# revision 3
# baseline (speedup 1.0000x reference)
"""Multi-head self-attention (causal) Trainium2 Bass kernel, 8-core SPMD.

Problem: B=2, S=2048, D_IN=1024, D_OUT=1024, H=16 heads (hd=64), causal
softmax, out-proj with bias.

Sharding: core c -> (batch b = c // 4, head-group g = c % 4). Each core
computes 4 heads of one batch: data-parallel over b, tensor-parallel over
heads (W_q/W_k/W_v column shards, W_o row shard). Host sums the 4 partial
out-proj results per batch and adds b_o.

On-core layout is fully transposed (feature-major):
  xT   [D_IN, S]                (host pre-transposes x[b])
  Qt,Kt[hd*4, S]  = W^T @ xT    (pair-major: 2 sbuf tiles of [128, S])
  V    [S, hd*4]  (normal orientation, ones column appended per head)
  St   [k, q] scores transposed; Pt = exp(St/8) in bf16
  ctxT [hd*4, S] with softmax denominator from the ones column
  outT [D_OUT, S] partial = Wo_shard^T @ ctxT  (host transposes back)

All matmul operands bf16 (fp32 PSUM accumulate); softmax in fp32.
"""

import numpy as np
import ml_dtypes

import concourse.bass as bass
import concourse.bacc as bacc
import concourse.tile as tile
import concourse.mybir as mybir
from concourse.bass_utils import run_bass_kernel_spmd

N_CORES = 8
B, S, D_IN, D_OUT, H = 2, 2048, 1024, 1024, 16
H_LOC = 4  # heads per core
HD = 64
DH = H_LOC * HD  # 256 = d_out shard per core
KI = D_IN // 128  # 8 contraction chunks
NQ = S // 512  # 4 seq chunks of 512
NB = S // 128  # 16 seq blocks of 128
SCALE = 1.0 / np.sqrt(np.float32(HD))  # 0.125

BF16 = mybir.dt.bfloat16
F32 = mybir.dt.float32
EXP = mybir.ActivationFunctionType.Exp
COPY = mybir.ActivationFunctionType.Copy


def _build_body(nc, tc, xT_d, wq_d, wk_d, wv_d, wo_d, outT_d):
    from contextlib import ExitStack

    ctx = ExitStack()
    const = ctx.enter_context(tc.tile_pool(name="const", bufs=1))
    mm_ps = ctx.enter_context(tc.tile_pool(name="mm_ps", bufs=2, space="PSUM"))
    score_ps = ctx.enter_context(tc.tile_pool(name="score_ps", bufs=4, space="PSUM"))
    ctx_ps = ctx.enter_context(tc.tile_pool(name="ctx_ps", bufs=2, space="PSUM"))
    pt_pool = ctx.enter_context(tc.tile_pool(name="pt", bufs=4))
    ob_pool = ctx.enter_context(tc.tile_pool(name="ob", bufs=3))
    r_pool = ctx.enter_context(tc.tile_pool(name="r", bufs=2))
    r64_pool = ctx.enter_context(tc.tile_pool(name="r64", bufs=2))

    # ---- resident SBUF tensors ----
    xT_s = const.tile([128, KI, S], BF16)
    wq_s = const.tile([128, KI, DH], BF16)
    wk_s = const.tile([128, KI, DH], BF16)
    wv_s = const.tile([128, KI, DH], BF16)
    wo_s = const.tile([128, 2, D_OUT], BF16)
    qt_s = const.tile([128, 2, S], BF16)  # [64*sub + hd, pair, q]
    kt_s = const.tile([128, 2, S], BF16)
    v_s = const.tile([128, NB, H_LOC, HD + 1], BF16)  # ones col at [..., HD]
    ctxT_s = const.tile([128, 2, S], BF16)
    tri_mask = const.tile([128, 128], F32)

    # ---- input DMAs ----
    for i in range(KI):
        nc.sync.dma_start(out=xT_s[:, i, :], in_=xT_d[128 * i : 128 * (i + 1), :])
        nc.sync.dma_start(out=wq_s[:, i, :], in_=wq_d[128 * i : 128 * (i + 1), :])
        nc.sync.dma_start(out=wk_s[:, i, :], in_=wk_d[128 * i : 128 * (i + 1), :])
        nc.sync.dma_start(out=wv_s[:, i, :], in_=wv_d[128 * i : 128 * (i + 1), :])
    for c in range(2):
        nc.sync.dma_start(out=wo_s[:, c, :], in_=wo_d[128 * c : 128 * (c + 1), :])

    # ---- constants: causal triangle mask (k_local > q_local -> -1e9) ----
    nc.vector.memset(tri_mask, 0.0)
    nc.gpsimd.affine_select(
        out=tri_mask,
        in_=tri_mask,
        compare_op=mybir.AluOpType.is_ge,
        fill=-1e9,
        base=0,
        pattern=[[1, 128]],
        channel_multiplier=-1,
    )
    # ones column of V (denominator trick)
    nc.vector.memset(v_s[:, :, :, HD : HD + 1], 1.0)

    # ---- QKV projections ----
    # Qt/Kt: out[do_chunk(128), q] = W[:, do_chunk]^T @ xT
    for w_s, dst in ((wq_s, qt_s), (wk_s, kt_s)):
        for m in range(2):  # head pair (output row chunk)
            for s4 in range(NQ):
                ps = mm_ps.tile([128, 512], F32, tag="mm")
                for ki in range(KI):
                    nc.tensor.matmul(
                        ps,
                        lhsT=w_s[:, ki, 128 * m : 128 * (m + 1)],
                        rhs=xT_s[:, ki, 512 * s4 : 512 * (s4 + 1)],
                        start=(ki == 0),
                        stop=(ki == KI - 1),
                    )
                nc.vector.tensor_copy(
                    out=qt_s[:, m, 512 * s4 : 512 * (s4 + 1)]
                    if dst is qt_s
                    else kt_s[:, m, 512 * s4 : 512 * (s4 + 1)],
                    in_=ps,
                )
    # V (normal orientation): out[s_block(128), do(256)] = x @ Wv
    for sb in range(NB):
        ps = mm_ps.tile([128, 256], F32, tag="mm")
        for ki in range(KI):
            nc.tensor.matmul(
                ps,
                lhsT=xT_s[:, ki, 128 * sb : 128 * (sb + 1)],
                rhs=wv_s[:, ki, :],
                start=(ki == 0),
                stop=(ki == KI - 1),
            )
        nc.vector.tensor_copy(
            out=v_s[:, sb, :, 0:HD],
            in_=ps.rearrange("p (h d) -> p h d", h=H_LOC),
        )

    # ---- attention (per head pair, q chunks of 512, transposed layout) ----
    for pair in range(2):
        for jj in range(NQ):
            q0 = 512 * jj
            nkb = 4 * (jj + 1)
            cps = [ctx_ps.tile([HD + 1, 512], F32, tag="ctx", name=f"cp{i}") for i in range(2)]
            for kb in range(nkb):
                d = kb - 4 * jj
                off = max(0, 128 * d)
                sps = [score_ps.tile([128, 512], F32, tag="sc", name=f"sp{i}") for i in range(2)]
                # paired scores matmuls (row groups 0-1 / 2-3 run concurrently)
                for i in range(2):
                    nc.tensor.matmul(
                        sps[i][:, off:512],
                        lhsT=kt_s[64 * i : 64 * i + 64, pair, 128 * kb : 128 * (kb + 1)],
                        rhs=qt_s[64 * i : 64 * i + 64, pair, q0 + off : q0 + 512],
                        start=True,
                        stop=True,
                    )
                for i in range(2):
                    h = 2 * pair + i
                    if d >= 0:
                        nc.vector.tensor_add(
                            sps[i][:, off : off + 128],
                            sps[i][:, off : off + 128],
                            tri_mask,
                        )
                    pt = pt_pool.tile([128, 512], BF16, tag="pt")
                    nc.scalar.activation(
                        out=pt[:, off:512],
                        in_=sps[i][:, off:512],
                        func=EXP,
                        scale=float(SCALE),
                    )
                    nc.tensor.matmul(
                        cps[i][:, off:512],
                        lhsT=v_s[:, kb, h, :],
                        rhs=pt[:, off:512],
                        start=(kb == 0),
                        stop=(kb == nkb - 1),
                    )
            # normalize: rows 0..63 ctx, row 64 = denominator
            for i in range(2):
                r = r_pool.tile([1, 512], F32, tag="r")
                nc.vector.reciprocal(out=r, in_=cps[i][HD : HD + 1, :])
                r64 = r64_pool.tile([64, 512], F32, tag="r64")
                nc.gpsimd.partition_broadcast(r64, r)
                nc.vector.tensor_mul(
                    out=ctxT_s[64 * i : 64 * i + 64, pair, q0 : q0 + 512],
                    in0=cps[i][0:HD, :],
                    in1=r64,
                )

    # ---- out projection: outT[o_chunk, q] = Wo^T @ ctxT ----
    for s4 in range(NQ):
        for m in range(8):
            op = mm_ps.tile([128, 512], F32, tag="mm")
            for c in range(2):
                nc.tensor.matmul(
                    op,
                    lhsT=wo_s[:, c, 128 * m : 128 * (m + 1)],
                    rhs=ctxT_s[:, c, 512 * s4 : 512 * (s4 + 1)],
                    start=(c == 0),
                    stop=(c == 1),
                )
            ob = ob_pool.tile([128, 512], F32, tag="ob")
            if m % 2 == 0:
                nc.vector.tensor_copy(out=ob, in_=op)
            else:
                nc.scalar.copy(out=ob, in_=op)
            nc.sync.dma_start(
                out=outT_d[128 * m : 128 * (m + 1), 512 * s4 : 512 * (s4 + 1)],
                in_=ob,
            )

    ctx.close()


_CACHED_NC = None


def _get_nc():
    global _CACHED_NC
    if _CACHED_NC is not None:
        return _CACHED_NC
    nc = bacc.Bacc(
        "TRN2", target_bir_lowering=False, debug=False, num_devices=N_CORES
    )
    xT_d = nc.dram_tensor("xT", [D_IN, S], BF16, kind="ExternalInput").ap()
    wq_d = nc.dram_tensor("wq", [D_IN, DH], BF16, kind="ExternalInput").ap()
    wk_d = nc.dram_tensor("wk", [D_IN, DH], BF16, kind="ExternalInput").ap()
    wv_d = nc.dram_tensor("wv", [D_IN, DH], BF16, kind="ExternalInput").ap()
    wo_d = nc.dram_tensor("wo", [DH, D_OUT], BF16, kind="ExternalInput").ap()
    outT_d = nc.dram_tensor("outT", [D_OUT, S], F32, kind="ExternalOutput").ap()
    with tile.TileContext(nc) as tc:
        _build_body(nc, tc, xT_d, wq_d, wk_d, wv_d, wo_d, outT_d)
    nc.compile()
    _CACHED_NC = nc
    return nc


def _make_in_maps(x, W_q, W_k, W_v, W_o):
    bf = ml_dtypes.bfloat16
    in_maps = []
    xT = [np.ascontiguousarray(x[b].T).astype(bf) for b in range(B)]
    for c in range(N_CORES):
        b, g = c // 4, c % 4
        sl = slice(DH * g, DH * (g + 1))
        in_maps.append(
            {
                "xT": xT[b],
                "wq": np.ascontiguousarray(W_q[:, sl]).astype(bf),
                "wk": np.ascontiguousarray(W_k[:, sl]).astype(bf),
                "wv": np.ascontiguousarray(W_v[:, sl]).astype(bf),
                "wo": np.ascontiguousarray(W_o[sl, :]).astype(bf),
            }
        )
    return in_maps


def run_cores(x, W_q, W_k, W_v, W_o, **spmd_kwargs):
    """Compile (cached), run on 8 cores, return raw results object."""
    nc = _get_nc()
    in_maps = _make_in_maps(x, W_q, W_k, W_v, W_o)
    return run_bass_kernel_spmd(
        nc, in_maps, core_ids=list(range(N_CORES)), **spmd_kwargs
    )


def gather(results, b_o):
    out = np.empty((B, S, D_OUT), np.float32)
    for b in range(B):
        acc = results[4 * b]["outT"].astype(np.float32).copy()
        for g in range(1, 4):
            acc += results[4 * b + g]["outT"]
        out[b] = acc.T + b_o.astype(np.float32)[None, :]
    return out


def kernel(x, W_q, W_k, W_v, W_o, b_o):
    x = np.asarray(x)
    res = run_cores(
        x, np.asarray(W_q), np.asarray(W_k), np.asarray(W_v), np.asarray(W_o)
    )
    return gather(res.results, np.asarray(b_o))


# revision 5
# speedup vs baseline: 1.2879x; 1.2879x over previous
"""Multi-head self-attention (causal) Trainium2 Bass kernel, 8-core SPMD.

Problem: B=2, S=2048, D_IN=1024, D_OUT=1024, H=16 heads (hd=64), causal
softmax, out-proj with bias.

Sharding: core c -> (batch b = c // 4, head-group g = c % 4). Each core
computes 4 heads of one batch: data-parallel over b, tensor-parallel over
heads (W_q/W_k/W_v column shards, W_o row shard). Host sums the 4 partial
out-proj results per batch and adds b_o.

On-core layout is fully transposed (feature-major):
  xT   [D_IN, S]                (host pre-transposes x[b])
  Qt,Kt[hd*4, S]  = W^T @ xT    (pair-major: 2 sbuf tiles of [128, S])
  V    [S, hd*4]  (normal orientation, ones column appended per head)
  St   [k, q] scores transposed; Pt = exp(St/8) in bf16
  ctxT [hd*4, S] with softmax denominator from the ones column
  outT [D_OUT, S] partial = Wo_shard^T @ ctxT  (host transposes back)

All matmul operands bf16 (fp32 PSUM accumulate); softmax in fp32.
Scores for a head pair are packed: head0 on PE row-groups 0-1, head1 on
2-3 (concurrent matmuls), psum tiles merged so one ACT exp call covers
both heads of a k-block.
"""

import numpy as np
import ml_dtypes

import concourse.bass as bass
import concourse.bacc as bacc
import concourse.tile as tile
import concourse.mybir as mybir
from concourse.bass_utils import run_bass_kernel_spmd

N_CORES = 8
B, S, D_IN, D_OUT, H = 2, 2048, 1024, 1024, 16
H_LOC = 4  # heads per core
HD = 64
DH = H_LOC * HD  # 256 = d_out shard per core
KI = D_IN // 128  # 8 contraction chunks
NQ = S // 512  # 4 seq chunks of 512
NB = S // 128  # 16 seq blocks of 128
SCALE = 1.0 / np.sqrt(np.float32(HD))  # 0.125

BF16 = mybir.dt.bfloat16
F32 = mybir.dt.float32
EXP = mybir.ActivationFunctionType.Exp


def _pair_view(ap2d):
    """[128, 1024] tile -> [128, 2, 512] (head-major) view."""
    return ap2d.rearrange("p (h q) -> p h q", h=2)


def _build_body(nc, tc, xT_d, wq_d, wk_d, wv_d, wo_d, outT_d):
    from contextlib import ExitStack

    ctx = ExitStack()
    const = ctx.enter_context(tc.tile_pool(name="const", bufs=1))
    # PSUM: "sc" tag [128,1024] x2 bufs = 4 banks; "ctx" [65,512] x4 = 4 banks
    sc_ps = ctx.enter_context(tc.tile_pool(name="sc_ps", bufs=2, space="PSUM"))
    ctx_ps = ctx.enter_context(tc.tile_pool(name="ctx_ps", bufs=4, space="PSUM"))
    pt_pool = ctx.enter_context(tc.tile_pool(name="pt", bufs=4))
    ob_pool = ctx.enter_context(tc.tile_pool(name="ob", bufs=3))
    r_pool = ctx.enter_context(tc.tile_pool(name="r", bufs=4))
    r64_pool = ctx.enter_context(tc.tile_pool(name="r64", bufs=4))

    # ---- resident SBUF tensors ----
    xT_s = const.tile([128, KI, S], BF16)
    wq_s = const.tile([128, KI, DH], BF16)
    wk_s = const.tile([128, KI, DH], BF16)
    wv_s = const.tile([128, KI, DH], BF16)
    wo_s = const.tile([128, 2, D_OUT], BF16)
    qt_s = const.tile([128, 2, S], BF16)  # [64*sub + hd, pair, q]
    kt_s = const.tile([128, 2, S], BF16)
    v_s = const.tile([128, NB, H_LOC, HD + 1], BF16)  # ones col at [..., HD]
    ctxT_s = const.tile([128, 2, S], BF16)
    tri_mask = const.tile([128, 128], F32)

    # ---- input DMAs ----
    for i in range(KI):
        nc.sync.dma_start(out=xT_s[:, i, :], in_=xT_d[128 * i : 128 * (i + 1), :])
        nc.sync.dma_start(out=wq_s[:, i, :], in_=wq_d[128 * i : 128 * (i + 1), :])
        nc.sync.dma_start(out=wk_s[:, i, :], in_=wk_d[128 * i : 128 * (i + 1), :])
        nc.sync.dma_start(out=wv_s[:, i, :], in_=wv_d[128 * i : 128 * (i + 1), :])
    for c in range(2):
        nc.sync.dma_start(out=wo_s[:, c, :], in_=wo_d[128 * c : 128 * (c + 1), :])

    # ---- constants: causal triangle mask (k_local > q_local -> -1e9) ----
    nc.vector.memset(tri_mask, 0.0)
    nc.gpsimd.affine_select(
        out=tri_mask,
        in_=tri_mask,
        compare_op=mybir.AluOpType.is_ge,
        fill=-1e9,
        base=0,
        pattern=[[1, 128]],
        channel_multiplier=-1,
    )
    tri2 = bass.AP(  # [128, 2, 128] view, head dim broadcast (step 0)
        tensor=tri_mask.tensor,
        offset=tri_mask.offset,
        ap=[list(tri_mask.ap[0]), [0, 2], list(tri_mask.ap[1])],
    )
    # ones column of V (denominator trick)
    nc.vector.memset(v_s[:, :, :, HD : HD + 1], 1.0)

    # ---- V projection (normal orientation): out[s_block, do] = x @ Wv ----
    for sb in range(NB):
        ps = sc_ps.tile([128, 256], F32, tag="sc", name="psv")
        for ki in range(KI):
            nc.tensor.matmul(
                ps,
                lhsT=xT_s[:, ki, 128 * sb : 128 * (sb + 1)],
                rhs=wv_s[:, ki, :],
                start=(ki == 0),
                stop=(ki == KI - 1),
            )
        nc.vector.tensor_copy(
            out=v_s[:, sb, :, 0:HD],
            in_=ps.rearrange("p (h d) -> p h d", h=H_LOC),
        )

    for pair in range(2):
        # ---- Q/K projections for this pair ----
        for w_s, dst in ((wq_s, qt_s), (wk_s, kt_s)):
            for s4 in range(NQ):
                ps = sc_ps.tile([128, 512], F32, tag="sc", name="psqk")
                for ki in range(KI):
                    nc.tensor.matmul(
                        ps,
                        lhsT=w_s[:, ki, 128 * pair : 128 * (pair + 1)],
                        rhs=xT_s[:, ki, 512 * s4 : 512 * (s4 + 1)],
                        start=(ki == 0),
                        stop=(ki == KI - 1),
                    )
                nc.vector.tensor_copy(
                    out=dst[:, pair, 512 * s4 : 512 * (s4 + 1)], in_=ps
                )

        # ---- attention for this pair (q chunks of 512, transposed) ----
        for jj in range(NQ):
            q0 = 512 * jj
            nkb = 4 * (jj + 1)
            cps = [
                ctx_ps.tile([HD + 1, 512], F32, tag="ctx", name=f"cp{i}")
                for i in range(2)
            ]
            for kb in range(nkb):
                d = kb - 4 * jj
                off = max(0, 128 * d)
                sp = sc_ps.tile([128, 1024], F32, tag="sc", name="sp")
                spv = _pair_view(sp)
                # paired scores matmuls (row groups 0-1 / 2-3 concurrent)
                for i in range(2):
                    nc.tensor.matmul(
                        spv[:, i, off:512],
                        lhsT=kt_s[
                            64 * i : 64 * i + 64, pair, 128 * kb : 128 * (kb + 1)
                        ],
                        rhs=qt_s[64 * i : 64 * i + 64, pair, q0 + off : q0 + 512],
                        start=True,
                        stop=True,
                    )
                if d >= 0:  # diagonal block: triangular causal mask, both heads
                    nc.vector.tensor_add(
                        spv[:, :, off : off + 128],
                        spv[:, :, off : off + 128],
                        tri2,
                    )
                pt = pt_pool.tile([128, 1024], BF16, tag="pt")
                ptv = _pair_view(pt)
                nc.scalar.activation(
                    out=ptv[:, :, off:512],
                    in_=spv[:, :, off:512],
                    func=EXP,
                    scale=float(SCALE),
                )
                for i in range(2):
                    h = 2 * pair + i
                    nc.tensor.matmul(
                        cps[i][:, off:512],
                        lhsT=v_s[:, kb, h, :],
                        rhs=ptv[:, i, off:512],
                        start=(kb == 0),
                        stop=(kb == nkb - 1),
                    )
            # normalize: rows 0..63 ctx, row 64 = denominator
            for i in range(2):
                d0 = r_pool.tile([1, 512], F32, tag="d0")
                nc.vector.tensor_copy(out=d0, in_=cps[i][HD : HD + 1, :])
                r = r_pool.tile([1, 512], F32, tag="r")
                nc.vector.reciprocal_approx_fast(out=r, in_=d0)
                r64 = r64_pool.tile([64, 512], F32, tag="r64")
                nc.gpsimd.partition_broadcast(r64, r)
                nc.vector.tensor_mul(
                    out=ctxT_s[64 * i : 64 * i + 64, pair, q0 : q0 + 512],
                    in0=cps[i][0:HD, :],
                    in1=r64,
                )

    # ---- out projection: outT[o_chunk, q] = Wo^T @ ctxT ----
    for s4 in range(NQ):
        for m in range(8):
            op = sc_ps.tile([128, 512], F32, tag="sc", name="pso")
            for c in range(2):
                nc.tensor.matmul(
                    op,
                    lhsT=wo_s[:, c, 128 * m : 128 * (m + 1)],
                    rhs=ctxT_s[:, c, 512 * s4 : 512 * (s4 + 1)],
                    start=(c == 0),
                    stop=(c == 1),
                )
            ob = ob_pool.tile([128, 512], F32, tag="ob")
            if m % 2 == 0:
                nc.vector.tensor_copy(out=ob, in_=op)
            else:
                nc.scalar.copy(out=ob, in_=op)
            nc.sync.dma_start(
                out=outT_d[128 * m : 128 * (m + 1), 512 * s4 : 512 * (s4 + 1)],
                in_=ob,
            )

    ctx.close()


_CACHED_NC = None


def _get_nc():
    global _CACHED_NC
    if _CACHED_NC is not None:
        return _CACHED_NC
    nc = bacc.Bacc(
        "TRN2", target_bir_lowering=False, debug=False, num_devices=N_CORES
    )
    xT_d = nc.dram_tensor("xT", [D_IN, S], BF16, kind="ExternalInput").ap()
    wq_d = nc.dram_tensor("wq", [D_IN, DH], BF16, kind="ExternalInput").ap()
    wk_d = nc.dram_tensor("wk", [D_IN, DH], BF16, kind="ExternalInput").ap()
    wv_d = nc.dram_tensor("wv", [D_IN, DH], BF16, kind="ExternalInput").ap()
    wo_d = nc.dram_tensor("wo", [DH, D_OUT], BF16, kind="ExternalInput").ap()
    outT_d = nc.dram_tensor("outT", [D_OUT, S], F32, kind="ExternalOutput").ap()
    with tile.TileContext(nc) as tc:
        _build_body(nc, tc, xT_d, wq_d, wk_d, wv_d, wo_d, outT_d)
    nc.compile()
    _CACHED_NC = nc
    return nc


def _make_in_maps(x, W_q, W_k, W_v, W_o):
    bf = ml_dtypes.bfloat16
    in_maps = []
    xT = [np.ascontiguousarray(x[b].T).astype(bf) for b in range(B)]
    for c in range(N_CORES):
        b, g = c // 4, c % 4
        sl = slice(DH * g, DH * (g + 1))
        in_maps.append(
            {
                "xT": xT[b],
                "wq": np.ascontiguousarray(W_q[:, sl]).astype(bf),
                "wk": np.ascontiguousarray(W_k[:, sl]).astype(bf),
                "wv": np.ascontiguousarray(W_v[:, sl]).astype(bf),
                "wo": np.ascontiguousarray(W_o[sl, :]).astype(bf),
            }
        )
    return in_maps


def run_cores(x, W_q, W_k, W_v, W_o, **spmd_kwargs):
    """Compile (cached), run on 8 cores, return raw results object."""
    nc = _get_nc()
    in_maps = _make_in_maps(x, W_q, W_k, W_v, W_o)
    return run_bass_kernel_spmd(
        nc, in_maps, core_ids=list(range(N_CORES)), **spmd_kwargs
    )


def gather(results, b_o):
    out = np.empty((B, S, D_OUT), np.float32)
    for b in range(B):
        acc = results[4 * b]["outT"].astype(np.float32).copy()
        for g in range(1, 4):
            acc += results[4 * b + g]["outT"]
        out[b] = acc.T + b_o.astype(np.float32)[None, :]
    return out


def kernel(x, W_q, W_k, W_v, W_o, b_o):
    x = np.asarray(x)
    res = run_cores(
        x, np.asarray(W_q), np.asarray(W_k), np.asarray(W_v), np.asarray(W_o)
    )
    return gather(res.results, np.asarray(b_o))
